# revision 16
# baseline (speedup 1.0000x reference)
"""Trainium2 Bass kernel for OCGatherEnergyCorrFac (segment_reduce).

out[i] = sum_{j: same (event, sid) as i} energy[j] * corr[j], with corr
zeroed for noise hits (sid == -1).

Strategy (8 NeuronCores):
- Host splits every event into 8 equal pieces; core k gets piece k of every
  event -> perfectly balanced shards and a single uniform program (per-event
  chunk column counts are compile-time constants shared by all cores).
- Scatter: per 128-hit column, one-hot matmul accumulated into a per-event
  PSUM table [128 lo x 17 q] (sid_p1 = q*128 + lo, sid_p1 in [0, 2049)).
- Per-core partial tables (8 x 2176 entries) are AllReduce-summed across the
  8 cores (68KB).
- Gather: indirect DMA from the reduced table using per-hit flat indices
  (idx = lo*136 + 17*e + q), computed on-device and kept SBUF-resident.
"""

import sys

sys.path.insert(0, "/opt/trn_rl_repo")

import functools

import numpy as np

import concourse.bass as bass
import concourse.mybir as mybir
import concourse.tile as tile
from concourse import bacc
from concourse.bass_utils import run_bass_kernel_spmd

NCORES = 8
B = 8                  # events
SEV = 2049             # sid_p1 range per event
QW = 17                # q = sid_p1 >> 7 in [0, 17)
TBL_COLS = B * QW      # 136
TBL_N = 128 * TBL_COLS  # 17408; flat idx = lo*136 + 17*e + q
BW = 512               # columns per processing block
GW = 16                # columns per one-hot group build
GB = 256               # gather block columns
F32 = mybir.dt.float32
BF16 = mybir.dt.bfloat16
OH_DT = BF16           # dtype of one-hot operands fed to the PE
I32 = mybir.dt.int32
AOP = mybir.AluOpType


def _layout(row_splits):
    rs = np.asarray(row_splits, dtype=np.int64).ravel()
    lens = np.diff(rs)                      # [B]
    pl = -(-lens // NCORES)                 # piece length per core (ceil)
    c_e = -(-pl // 128)                     # columns per chunk (ceil)
    return rs, lens, pl, c_e


@functools.lru_cache(maxsize=4)
def _build(c_e_tuple):
    """Build + compile the uniform per-core program for chunk column counts."""
    c_e = list(c_e_tuple)
    c_total = int(sum(c_e))
    assert c_total > 0

    nc = bacc.Bacc("TRN2", target_bir_lowering=False, debug=False,
                   num_devices=NCORES)
    sid_d = nc.dram_tensor("sid", [128, c_total], I32, kind="ExternalInput").ap()
    corr_d = nc.dram_tensor("corr", [128, c_total], F32, kind="ExternalInput").ap()
    en_d = nc.dram_tensor("energy", [128, c_total], F32, kind="ExternalInput").ap()
    out_d = nc.dram_tensor("out", [8, 16 * c_total], F32, kind="ExternalOutput").ap()
    tloc = nc.dram_tensor("tloc", [1, TBL_N], F32).ap()
    tglob = nc.dram_tensor("tglob", [1, TBL_N], F32).ap()

    with tile.TileContext(nc) as tc:
        with (
            tc.tile_pool(name="const", bufs=1) as constp,
            tc.tile_pool(name="io", bufs=2) as iop,
            tc.tile_pool(name="work", bufs=2) as workp,
            tc.tile_pool(name="oh", bufs=3) as ohp,
            tc.tile_pool(name="og", bufs=2) as ogp,
            tc.tile_pool(name="seg", bufs=1) as segp,
            tc.tile_pool(name="tbl", bufs=1) as tblp,
            tc.tile_pool(name="psum", bufs=1, space="PSUM") as psump,
        ):
            # constants
            iota_lo = constp.tile([128, 128], OH_DT, tag="il_f")
            nc.gpsimd.iota(iota_lo[:], pattern=[[1, 128]], base=0,
                           channel_multiplier=0,
                           allow_small_or_imprecise_dtypes=True)
            iota_q = constp.tile([128, QW], OH_DT, tag="iq_f")
            nc.gpsimd.iota(iota_q[:], pattern=[[1, QW]], base=0,
                           channel_multiplier=0,
                           allow_small_or_imprecise_dtypes=True)

            seg_sb = segp.tile([128, c_total], mybir.dt.int16, tag="seg")
            table_sb = tblp.tile([128, TBL_COLS], F32, tag="tbl")
            nc.gpsimd.memset(table_sb[:], 0.0)

            # ---------------- phase A: scatter ----------------
            seg_instrs = []
            col0 = 0
            for e in range(B):
                ce = c_e[e]
                if ce == 0:
                    continue
                psum_e = psump.tile([128, QW], F32, tag=f"ev{e}")
                first = True
                for b0 in range(0, ce, BW):
                    w = min(BW, ce - b0)
                    cs = col0 + b0  # global column offset of this block
                    st = iop.tile([128, BW], I32, tag="sid")
                    ct = iop.tile([128, BW], F32, tag="corr")
                    et = iop.tile([128, BW], F32, tag="en")
                    nc.sync.dma_start(out=st[:, :w], in_=sid_d[:, cs:cs + w])
                    nc.sync.dma_start(out=ct[:, :w], in_=corr_d[:, cs:cs + w])
                    nc.sync.dma_start(out=et[:, :w], in_=en_d[:, cs:cs + w])

                    lo_i = workp.tile([128, BW], I32, tag="lo_i")
                    q_i = workp.tile([128, BW], I32, tag="q_i")
                    lo_f = workp.tile([128, BW], F32, tag="lo_f")
                    q_f = workp.tile([128, BW], F32, tag="q_f")
                    ve = workp.tile([128, BW], F32, tag="ve")
                    v = workp.tile([128, BW], F32, tag="v")

                    vm = workp.tile([128, BW], F32, tag="vm")
                    s_i = workp.tile([128, BW], I32, tag="s_i")
                    nc.vector.tensor_scalar_add(s_i[:, :w], st[:, :w], 1)
                    nc.vector.tensor_scalar(out=lo_i[:, :w], in0=s_i[:, :w],
                                            scalar1=127, scalar2=None,
                                            op0=AOP.bitwise_and)
                    nc.vector.tensor_scalar(out=q_i[:, :w], in0=s_i[:, :w],
                                            scalar1=7, scalar2=None,
                                            op0=AOP.logical_shift_right)
                    nc.vector.tensor_copy(lo_f[:, :w], lo_i[:, :w])
                    nc.vector.tensor_copy(q_f[:, :w], q_i[:, :w])
                    nc.gpsimd.tensor_tensor(out=ve[:, :w], in0=ct[:, :w],
                                            in1=et[:, :w], op=AOP.mult)
                    # v = (sid >= 0) * energy * corr
                    nc.gpsimd.tensor_scalar(out=vm[:, :w], in0=st[:, :w],
                                            scalar1=0, scalar2=None,
                                            op0=AOP.is_ge)
                    nc.gpsimd.tensor_tensor(out=v[:, :w], in0=vm[:, :w],
                                            in1=ve[:, :w], op=AOP.mult)
                    # seg index = lo*136 + q + 17*e  (int16, SBUF-resident)
                    seg32 = s_i
                    nc.vector.tensor_scalar(out=seg32[:, :w],
                                            in0=lo_i[:, :w], scalar1=TBL_COLS,
                                            scalar2=17 * e, op0=AOP.mult,
                                            op1=AOP.add)
                    nc.vector.tensor_tensor(out=seg32[:, :w],
                                            in0=seg32[:, :w],
                                            in1=q_i[:, :w], op=AOP.add)
                    si = nc.vector.tensor_copy(seg_sb[:, cs:cs + w],
                                               seg32[:, :w])
                    seg_instrs.append((cs, cs + w, si))

                    for g0 in range(0, w, GW):
                        gw = min(GW, w - g0)
                        oh = ohp.tile([128, GW * 128], OH_DT, tag="oh")
                        rhs = ohp.tile([128, GW * QW], OH_DT, tag="rhs")
                        for j in range(gw):
                            c = g0 + j
                            nc.vector.tensor_scalar(
                                out=oh[:, j * 128:(j + 1) * 128],
                                in0=iota_lo[:], scalar1=lo_f[:, c:c + 1],
                                scalar2=None, op0=AOP.is_equal)
                            nc.vector.tensor_scalar(
                                out=rhs[:, j * QW:(j + 1) * QW],
                                in0=iota_q[:], scalar1=q_f[:, c:c + 1],
                                scalar2=v[:, c:c + 1], op0=AOP.is_equal,
                                op1=AOP.mult)
                            last = (b0 + c == ce - 1)
                            nc.tensor.matmul(
                                psum_e[:],
                                lhsT=oh[:, j * 128:(j + 1) * 128],
                                rhs=rhs[:, j * QW:(j + 1) * QW],
                                start=first, stop=last)
                            first = False
                nc.vector.tensor_copy(table_sb[:, QW * e:QW * (e + 1)], psum_e[:])
                col0 += ce

            # table -> DRAM (flat order = lo*136 + 17e + q), then AllReduce
            nc.sync.dma_start(out=tloc[:], in_=table_sb[:])
            cc = nc.gpsimd.collective_compute(
                "AllReduce", AOP.add,
                replica_groups=[list(range(NCORES))],
                ins=[tloc[:]], outs=[tglob[:]])

            # ---------------- phase B: gather (Q7 ap_gather) ----------------
            from concourse.tile_rust import add_dep_helper
            tbl_bc = tblp.tile([128, TBL_N], F32, tag="tblbc")
            bi = nc.sync.dma_start(out=tbl_bc[:],
                                   in_=tglob.partition_broadcast(128))
            add_dep_helper(bi.ins, cc.ins, reason="bcast after allreduce")
            for b0 in range(0, c_total, GB):
                w = min(GB, c_total - b0)
                og = ogp.tile([128, 16 * GB], F32, tag="og")
                ag = nc.gpsimd.ap_gather(
                    out_ap=og[:, :16 * w], in_ap=tbl_bc[:],
                    idxs_ap=seg_sb[:, b0:b0 + w],
                    channels=128, num_elems=TBL_N, d=1, num_idxs=16 * w)
                add_dep_helper(ag.ins, bi.ins, reason="gather after bcast")
                for (a0, a1, si) in seg_instrs:
                    if a0 < b0 + w and b0 < a1:
                        add_dep_helper(ag.ins, si.ins, reason="gather after seg")
                # one row per 16-partition group holds the full gathered
                # sequence; ship rows 0,16,...,112 straight to DRAM
                di = nc.sync.dma_start(out=out_d[:, 16 * b0:16 * (b0 + w)],
                                       in_=og[0:128:16, :16 * w])
                add_dep_helper(di.ins, ag.ins, reason="out after gather")

    nc.compile()
    return nc, c_total


class _Runner:
    """Cached PJRT executor for a compiled Bass program (axon path).

    Replicates bass2jax.run_bass_via_pjrt but keeps the jitted callable so
    repeated executions reuse the compiled NEFF, and donates the previous
    call's outputs as the next call's output buffers (kernel writes every
    output element, so zero-init is not required).
    """

    def __init__(self, nc):
        import jax
        from jax.sharding import Mesh, PartitionSpec
        from jax.experimental.shard_map import shard_map
        from concourse import bass2jax
        from concourse.bass2jax import _bass_exec_p, install_neuronx_cc_hook

        install_neuronx_cc_hook()
        self.nc = nc
        self.jax = jax
        partition_name = (nc.partition_id_tensor.name
                          if nc.partition_id_tensor else None)
        in_names, out_names, out_avals = [], [], []
        for alloc in nc.m.functions[0].allocations:
            if not isinstance(alloc, mybir.MemoryLocationSet):
                continue
            name = alloc.memorylocations[0].name
            if alloc.kind == "ExternalInput":
                if name != partition_name:
                    in_names.append(name)
            elif alloc.kind == "ExternalOutput":
                out_names.append(name)
                out_avals.append(jax.core.ShapedArray(
                    tuple(alloc.tensor_shape), mybir.dt.np(alloc.dtype)))
        self.in_names = list(in_names)
        self.out_names = out_names
        self.out_avals = out_avals
        n_params = len(in_names)
        n_outs = len(out_avals)
        all_names = in_names + out_names
        if partition_name is not None:
            all_names = all_names + [partition_name]

        def _body(*args):
            operands = list(args)
            if partition_name is not None:
                operands.append(bass2jax.partition_id_tensor())
            outs = _bass_exec_p.bind(
                *operands,
                out_avals=tuple(out_avals),
                in_names=tuple(all_names),
                out_names=tuple(out_names),
                lowering_input_output_aliases=(),
                sim_require_finite=True,
                sim_require_nnan=True,
                nc=nc,
            )
            return tuple(outs)

        devices = jax.devices()[:NCORES]
        mesh = Mesh(np.asarray(devices), ("core",))
        in_specs = (PartitionSpec("core"),) * (n_params + n_outs)
        out_specs = (PartitionSpec("core"),) * n_outs
        self.donate = tuple(range(n_params, n_params + n_outs))
        self.fn = jax.jit(
            shard_map(_body, mesh=mesh, in_specs=in_specs,
                      out_specs=out_specs, check_rep=False),
            donate_argnums=self.donate, keep_unused=True)
        self._dev_in = None
        self._prev_out = None

    def load_inputs(self, in_maps):
        concat = [
            np.concatenate([np.asarray(in_maps[c][n]) for c in range(NCORES)],
                           axis=0)
            for n in self.in_names
        ]
        self._dev_in = [self.jax.device_put(a) for a in concat]
        self._prev_out = None

    def run_once(self):
        if self._prev_out is None:
            outs = [np.zeros((NCORES * a.shape[0], *a.shape[1:]), a.dtype)
                    for a in self.out_avals]
        else:
            outs = self._prev_out
        res = self.fn(*self._dev_in, *outs)
        self.jax.block_until_ready(res)
        self._prev_out = list(res)
        return res

    def results(self):
        res = self._prev_out
        return [
            {name: np.asarray(res[i]).reshape(NCORES, *self.out_avals[i].shape)[c]
             for i, name in enumerate(self.out_names)}
            for c in range(NCORES)
        ]


_RUNNERS = {}


def _get_runner(c_e_tuple):
    if c_e_tuple not in _RUNNERS:
        nc, c_total = _build(c_e_tuple)
        _RUNNERS[c_e_tuple] = (_Runner(nc), c_total)
    return _RUNNERS[c_e_tuple]


def _prep_core(pred_sid, pred_corr_factor, rechit_energy, rs, pl, c_e, c_total, k):
    sid = np.full((128, c_total), -1, dtype=np.int32)
    corr = np.zeros((128, c_total), dtype=np.float32)
    en = np.zeros((128, c_total), dtype=np.float32)
    col0 = 0
    for e in range(B):
        ce = int(c_e[e])
        if ce == 0:
            continue
        i0 = int(rs[e] + k * pl[e])
        i1 = int(min(rs[e] + (k + 1) * pl[e], rs[e + 1]))
        n = max(0, i1 - i0)
        pad = ce * 128
        if n > 0:
            blk = np.full(pad, -1, dtype=np.int32)
            blk[:n] = pred_sid[i0:i1, 0]
            sid[:, col0:col0 + ce] = blk.reshape(128, ce)
            blkf = np.zeros(pad, dtype=np.float32)
            blkf[:n] = pred_corr_factor[i0:i1, 0]
            corr[:, col0:col0 + ce] = blkf.reshape(128, ce)
            blkf = np.zeros(pad, dtype=np.float32)
            blkf[:n] = rechit_energy[i0:i1, 0]
            en[:, col0:col0 + ce] = blkf.reshape(128, ce)
        col0 += ce
    return {"sid": sid, "corr": corr, "energy": en}


def kernel(pred_sid, pred_corr_factor, rechit_energy, row_splits):
    pred_sid = np.asarray(pred_sid)
    pred_corr_factor = np.asarray(pred_corr_factor)
    rechit_energy = np.asarray(rechit_energy)
    row_splits = np.asarray(row_splits)

    rs, lens, pl, c_e = _layout(row_splits)
    runner, c_total = _get_runner(tuple(int(x) for x in c_e))

    in_maps = [
        _prep_core(pred_sid, pred_corr_factor, rechit_energy, rs, pl, c_e,
                   c_total, k)
        for k in range(NCORES)
    ]
    runner.load_inputs(in_maps)
    runner.run_once()
    results = runner.results()

    n = pred_sid.shape[0]
    out = np.zeros((n,), dtype=np.float32)
    for k in range(NCORES):
        og = results[k]["out"]  # [8, 16*c_total]
        o = og.reshape(8, c_total, 16).transpose(0, 2, 1).reshape(128, c_total)
        col0 = 0
        for e in range(B):
            ce = int(c_e[e])
            if ce == 0:
                continue
            i0 = int(rs[e] + k * pl[e])
            i1 = int(min(rs[e] + (k + 1) * pl[e], rs[e + 1]))
            nk = max(0, i1 - i0)
            if nk > 0:
                out[i0:i1] = o[:, col0:col0 + ce].reshape(-1)[:nk]
            col0 += ce
    return out[:, None]


# revision 17
# speedup vs baseline: 5.1472x; 5.1472x over previous
"""Trainium2 Bass kernel for OCGatherEnergyCorrFac (segment_reduce).

out[i] = sum_{j: same (event, sid) as i} energy[j] * corr[j], with corr
zeroed for noise hits (sid == -1).

Strategy (8 NeuronCores):
- Host splits every event into 8 equal pieces; core k gets piece k of every
  event -> perfectly balanced shards and a single uniform program (per-event
  chunk column counts are compile-time constants shared by all cores).
- Scatter: per 128-hit column, one-hot matmul accumulated into a per-event
  PSUM table [128 lo x 17 q] (sid_p1 = q*128 + lo, sid_p1 in [0, 2049)).
- Per-core partial tables (8 x 2176 entries) are AllReduce-summed across the
  8 cores (68KB).
- Gather: indirect DMA from the reduced table using per-hit flat indices
  (idx = lo*136 + 17*e + q), computed on-device and kept SBUF-resident.
"""

import sys

sys.path.insert(0, "/opt/trn_rl_repo")

import functools

import numpy as np

import concourse.bass as bass
import concourse.mybir as mybir
import concourse.tile as tile
from concourse import bacc
from concourse.bass_utils import run_bass_kernel_spmd

NCORES = 8
B = 8                  # events
SEV = 2049             # sid_p1 range per event
QW = 17                # q = sid_p1 >> 7 in [0, 17)
TBL_COLS = B * QW      # 136
TBL_N = 128 * TBL_COLS  # 17408; flat idx = lo*136 + 17*e + q
BW = 512               # columns per processing block
GW = 16                # columns per one-hot group build
GB = 512               # gather block columns
F32 = mybir.dt.float32
BF16 = mybir.dt.bfloat16
OH_DT = BF16           # dtype of one-hot operands fed to the PE
I32 = mybir.dt.int32
AOP = mybir.AluOpType


def _layout(row_splits):
    rs = np.asarray(row_splits, dtype=np.int64).ravel()
    lens = np.diff(rs)                      # [B]
    pl = -(-lens // NCORES)                 # piece length per core (ceil)
    c_e = -(-pl // 128)                     # columns per chunk (ceil)
    return rs, lens, pl, c_e


@functools.lru_cache(maxsize=4)
def _build(c_e_tuple):
    """Build + compile the uniform per-core program for chunk column counts."""
    c_e = list(c_e_tuple)
    c_total = int(sum(c_e))
    assert c_total > 0

    nc = bacc.Bacc("TRN2", target_bir_lowering=False, debug=False,
                   num_devices=NCORES)
    sid_d = nc.dram_tensor("sid", [128, c_total], I32, kind="ExternalInput").ap()
    corr_d = nc.dram_tensor("corr", [128, c_total], F32, kind="ExternalInput").ap()
    en_d = nc.dram_tensor("energy", [128, c_total], F32, kind="ExternalInput").ap()
    out_d = nc.dram_tensor("out", [8, 16 * c_total], F32, kind="ExternalOutput").ap()
    tloc = nc.dram_tensor("tloc", [1, TBL_N], F32).ap()
    tglob = nc.dram_tensor("tglob", [1, TBL_N], F32).ap()

    with tile.TileContext(nc) as tc:
        with (
            tc.tile_pool(name="const", bufs=1) as constp,
            tc.tile_pool(name="io", bufs=2) as iop,
            tc.tile_pool(name="work", bufs=2) as workp,
            tc.tile_pool(name="oh", bufs=3) as ohp,
            tc.tile_pool(name="og", bufs=1) as ogp,
            tc.tile_pool(name="seg", bufs=1) as segp,
            tc.tile_pool(name="tbl", bufs=1) as tblp,
            tc.tile_pool(name="psum", bufs=1, space="PSUM") as psump,
        ):
            # constants
            iota_lo = constp.tile([128, GW * 128], OH_DT, tag="il_f")
            nc.gpsimd.iota(iota_lo[:], pattern=[[0, GW], [1, 128]], base=0,
                           channel_multiplier=0,
                           allow_small_or_imprecise_dtypes=True)
            iota_q = constp.tile([128, QW], OH_DT, tag="iq_f")
            nc.gpsimd.iota(iota_q[:], pattern=[[1, QW]], base=0,
                           channel_multiplier=0,
                           allow_small_or_imprecise_dtypes=True)
            lo_b = None

            seg_sb = segp.tile([128, c_total], mybir.dt.int16, tag="seg")
            table_sb = tblp.tile([128, TBL_COLS], F32, tag="tbl")
            nc.gpsimd.memset(table_sb[:], 0.0)

            # ---------------- phase A: scatter ----------------
            seg_instrs = []
            col0 = 0
            for e in range(B):
                ce = c_e[e]
                if ce == 0:
                    continue
                psum_e = psump.tile([128, QW], F32, tag=f"ev{e}")
                first = True
                for b0 in range(0, ce, BW):
                    w = min(BW, ce - b0)
                    cs = col0 + b0  # global column offset of this block
                    st = iop.tile([128, BW], I32, tag="sid")
                    ct = iop.tile([128, BW], F32, tag="corr")
                    et = iop.tile([128, BW], F32, tag="en")
                    nc.sync.dma_start(out=st[:, :w], in_=sid_d[:, cs:cs + w])
                    nc.sync.dma_start(out=ct[:, :w], in_=corr_d[:, cs:cs + w])
                    nc.sync.dma_start(out=et[:, :w], in_=en_d[:, cs:cs + w])

                    lo_i = workp.tile([128, BW], I32, tag="lo_i")
                    q_i = workp.tile([128, BW], I32, tag="q_i")
                    lo_f = workp.tile([128, BW], F32, tag="lo_f")
                    lo_b = workp.tile([128, BW], OH_DT, tag="lo_b")
                    q_f = workp.tile([128, BW], F32, tag="q_f")
                    ve = workp.tile([128, BW], F32, tag="ve")
                    v = workp.tile([128, BW], F32, tag="v")

                    vm = workp.tile([128, BW], F32, tag="vm")
                    s_i = workp.tile([128, BW], I32, tag="s_i")
                    nc.vector.tensor_scalar_add(s_i[:, :w], st[:, :w], 1)
                    nc.vector.tensor_scalar(out=lo_i[:, :w], in0=s_i[:, :w],
                                            scalar1=127, scalar2=None,
                                            op0=AOP.bitwise_and)
                    nc.vector.tensor_scalar(out=q_i[:, :w], in0=s_i[:, :w],
                                            scalar1=7, scalar2=None,
                                            op0=AOP.logical_shift_right)
                    nc.vector.tensor_copy(lo_f[:, :w], lo_i[:, :w])
                    nc.scalar.copy(lo_b[:, :w], lo_i[:, :w])
                    nc.vector.tensor_copy(q_f[:, :w], q_i[:, :w])
                    nc.gpsimd.tensor_tensor(out=ve[:, :w], in0=ct[:, :w],
                                            in1=et[:, :w], op=AOP.mult)
                    # v = (sid >= 0) * energy * corr
                    nc.gpsimd.tensor_scalar(out=vm[:, :w], in0=st[:, :w],
                                            scalar1=0, scalar2=None,
                                            op0=AOP.is_ge)
                    nc.gpsimd.tensor_tensor(out=v[:, :w], in0=vm[:, :w],
                                            in1=ve[:, :w], op=AOP.mult)
                    # seg index = lo*136 + q + 17*e  (int16, SBUF-resident)
                    seg32 = s_i
                    nc.vector.tensor_scalar(out=seg32[:, :w],
                                            in0=lo_i[:, :w], scalar1=TBL_COLS,
                                            scalar2=17 * e, op0=AOP.mult,
                                            op1=AOP.add)
                    nc.vector.tensor_tensor(out=seg32[:, :w],
                                            in0=seg32[:, :w],
                                            in1=q_i[:, :w], op=AOP.add)
                    si = nc.vector.tensor_copy(seg_sb[:, cs:cs + w],
                                               seg32[:, :w])
                    seg_instrs.append((cs, cs + w, si))

                    for g0 in range(0, w, GW):
                        gw = min(GW, w - g0)
                        oh = ohp.tile([128, GW * 128], OH_DT, tag="oh")
                        rhs = ohp.tile([128, GW * QW], OH_DT, tag="rhs")
                        nc.vector.tensor_tensor(
                            out=oh[:].rearrange("p (g x) -> p g x", x=128)[:, :gw],
                            in0=lo_b[:, g0:g0 + gw].to_broadcast([128, gw, 128]),
                            in1=iota_lo[:].rearrange("p (g x) -> p g x", x=128)[:, :gw],
                            op=AOP.is_equal)
                        for j in range(gw):
                            c = g0 + j
                            nc.vector.tensor_scalar(
                                out=rhs[:, j * QW:(j + 1) * QW],
                                in0=iota_q[:], scalar1=q_f[:, c:c + 1],
                                scalar2=v[:, c:c + 1], op0=AOP.is_equal,
                                op1=AOP.mult)
                            last = (b0 + c == ce - 1)
                            nc.tensor.matmul(
                                psum_e[:],
                                lhsT=oh[:, j * 128:(j + 1) * 128],
                                rhs=rhs[:, j * QW:(j + 1) * QW],
                                start=first, stop=last)
                            first = False
                nc.vector.tensor_copy(table_sb[:, QW * e:QW * (e + 1)], psum_e[:])
                col0 += ce

            # table -> DRAM (flat order = lo*136 + 17e + q), then AllReduce
            nc.sync.dma_start(out=tloc[:], in_=table_sb[:])
            cc = nc.gpsimd.collective_compute(
                "AllReduce", AOP.add,
                replica_groups=[list(range(NCORES))],
                ins=[tloc[:]], outs=[tglob[:]])

            # ---------------- phase B: gather (Q7 ap_gather) ----------------
            from concourse.tile_rust import add_dep_helper
            tbl_bc = tblp.tile([128, TBL_N], F32, tag="tblbc")
            bi = nc.sync.dma_start(out=tbl_bc[:],
                                   in_=tglob.partition_broadcast(128))
            add_dep_helper(bi.ins, cc.ins, reason="bcast after allreduce")
            for b0 in range(0, c_total, GB):
                w = min(GB, c_total - b0)
                og = ogp.tile([128, 16 * GB], F32, tag="og")
                ag = nc.gpsimd.ap_gather(
                    out_ap=og[:, :16 * w], in_ap=tbl_bc[:],
                    idxs_ap=seg_sb[:, b0:b0 + w],
                    channels=128, num_elems=TBL_N, d=1, num_idxs=16 * w)
                add_dep_helper(ag.ins, bi.ins, reason="gather after bcast")
                for (a0, a1, si) in seg_instrs:
                    if a0 < b0 + w and b0 < a1:
                        add_dep_helper(ag.ins, si.ins, reason="gather after seg")
                # one row per 16-partition group holds the full gathered
                # sequence; ship rows 0,16,...,112 straight to DRAM
                di = nc.sync.dma_start(out=out_d[:, 16 * b0:16 * (b0 + w)],
                                       in_=og[0:128:16, :16 * w])
                add_dep_helper(di.ins, ag.ins, reason="out after gather")

    nc.compile()
    return nc, c_total


class _Runner:
    """Cached PJRT executor for a compiled Bass program (axon path).

    Replicates bass2jax.run_bass_via_pjrt but keeps the jitted callable so
    repeated executions reuse the compiled NEFF, and donates the previous
    call's outputs as the next call's output buffers (kernel writes every
    output element, so zero-init is not required).
    """

    def __init__(self, nc):
        import jax
        from jax.sharding import Mesh, PartitionSpec
        from jax.experimental.shard_map import shard_map
        from concourse import bass2jax
        from concourse.bass2jax import _bass_exec_p, install_neuronx_cc_hook

        install_neuronx_cc_hook()
        self.nc = nc
        self.jax = jax
        partition_name = (nc.partition_id_tensor.name
                          if nc.partition_id_tensor else None)
        in_names, out_names, out_avals = [], [], []
        for alloc in nc.m.functions[0].allocations:
            if not isinstance(alloc, mybir.MemoryLocationSet):
                continue
            name = alloc.memorylocations[0].name
            if alloc.kind == "ExternalInput":
                if name != partition_name:
                    in_names.append(name)
            elif alloc.kind == "ExternalOutput":
                out_names.append(name)
                out_avals.append(jax.core.ShapedArray(
                    tuple(alloc.tensor_shape), mybir.dt.np(alloc.dtype)))
        self.in_names = list(in_names)
        self.out_names = out_names
        self.out_avals = out_avals
        n_params = len(in_names)
        n_outs = len(out_avals)
        all_names = in_names + out_names
        if partition_name is not None:
            all_names = all_names + [partition_name]

        def _body(*args):
            operands = list(args)
            if partition_name is not None:
                operands.append(bass2jax.partition_id_tensor())
            outs = _bass_exec_p.bind(
                *operands,
                out_avals=tuple(out_avals),
                in_names=tuple(all_names),
                out_names=tuple(out_names),
                lowering_input_output_aliases=(),
                sim_require_finite=True,
                sim_require_nnan=True,
                nc=nc,
            )
            return tuple(outs)

        devices = jax.devices()[:NCORES]
        mesh = Mesh(np.asarray(devices), ("core",))
        in_specs = (PartitionSpec("core"),) * (n_params + n_outs)
        out_specs = (PartitionSpec("core"),) * n_outs
        self.donate = tuple(range(n_params, n_params + n_outs))
        self.fn = jax.jit(
            shard_map(_body, mesh=mesh, in_specs=in_specs,
                      out_specs=out_specs, check_rep=False),
            donate_argnums=self.donate, keep_unused=True)
        self._dev_in = None
        self._prev_out = None

    def load_inputs(self, in_maps):
        concat = [
            np.concatenate([np.asarray(in_maps[c][n]) for c in range(NCORES)],
                           axis=0)
            for n in self.in_names
        ]
        self._dev_in = [self.jax.device_put(a) for a in concat]
        self._prev_out = None

    def run_once(self):
        if self._prev_out is None:
            outs = [np.zeros((NCORES * a.shape[0], *a.shape[1:]), a.dtype)
                    for a in self.out_avals]
        else:
            outs = self._prev_out
        res = self.fn(*self._dev_in, *outs)
        self.jax.block_until_ready(res)
        self._prev_out = list(res)
        return res

    def results(self):
        res = self._prev_out
        return [
            {name: np.asarray(res[i]).reshape(NCORES, *self.out_avals[i].shape)[c]
             for i, name in enumerate(self.out_names)}
            for c in range(NCORES)
        ]


_RUNNERS = {}


def _get_runner(c_e_tuple):
    if c_e_tuple not in _RUNNERS:
        nc, c_total = _build(c_e_tuple)
        _RUNNERS[c_e_tuple] = (_Runner(nc), c_total)
    return _RUNNERS[c_e_tuple]


def _prep_core(pred_sid, pred_corr_factor, rechit_energy, rs, pl, c_e, c_total, k):
    sid = np.full((128, c_total), -1, dtype=np.int32)
    corr = np.zeros((128, c_total), dtype=np.float32)
    en = np.zeros((128, c_total), dtype=np.float32)
    col0 = 0
    for e in range(B):
        ce = int(c_e[e])
        if ce == 0:
            continue
        i0 = int(rs[e] + k * pl[e])
        i1 = int(min(rs[e] + (k + 1) * pl[e], rs[e + 1]))
        n = max(0, i1 - i0)
        pad = ce * 128
        if n > 0:
            blk = np.full(pad, -1, dtype=np.int32)
            blk[:n] = pred_sid[i0:i1, 0]
            sid[:, col0:col0 + ce] = blk.reshape(128, ce)
            blkf = np.zeros(pad, dtype=np.float32)
            blkf[:n] = pred_corr_factor[i0:i1, 0]
            corr[:, col0:col0 + ce] = blkf.reshape(128, ce)
            blkf = np.zeros(pad, dtype=np.float32)
            blkf[:n] = rechit_energy[i0:i1, 0]
            en[:, col0:col0 + ce] = blkf.reshape(128, ce)
        col0 += ce
    return {"sid": sid, "corr": corr, "energy": en}


def kernel(pred_sid, pred_corr_factor, rechit_energy, row_splits):
    pred_sid = np.asarray(pred_sid)
    pred_corr_factor = np.asarray(pred_corr_factor)
    rechit_energy = np.asarray(rechit_energy)
    row_splits = np.asarray(row_splits)

    rs, lens, pl, c_e = _layout(row_splits)
    runner, c_total = _get_runner(tuple(int(x) for x in c_e))

    in_maps = [
        _prep_core(pred_sid, pred_corr_factor, rechit_energy, rs, pl, c_e,
                   c_total, k)
        for k in range(NCORES)
    ]
    runner.load_inputs(in_maps)
    runner.run_once()
    results = runner.results()

    n = pred_sid.shape[0]
    out = np.zeros((n,), dtype=np.float32)
    for k in range(NCORES):
        og = results[k]["out"]  # [8, 16*c_total]
        o = og.reshape(8, c_total, 16).transpose(0, 2, 1).reshape(128, c_total)
        col0 = 0
        for e in range(B):
            ce = int(c_e[e])
            if ce == 0:
                continue
            i0 = int(rs[e] + k * pl[e])
            i1 = int(min(rs[e] + (k + 1) * pl[e], rs[e + 1]))
            nk = max(0, i1 - i0)
            if nk > 0:
                out[i0:i1] = o[:, col0:col0 + ce].reshape(-1)[:nk]
            col0 += ce
    return out[:, None]


# revision 24
# speedup vs baseline: 6.1283x; 1.1906x over previous
"""Trainium2 Bass kernel for OCGatherEnergyCorrFac (segment_reduce).

out[i] = sum_{j: same (event, sid) as i} energy[j] * corr[j], with corr
zeroed for noise hits (sid == -1).

Strategy (8 NeuronCores):
- Host splits every event into 8 equal pieces; core k gets piece k of every
  event -> perfectly balanced shards and a single uniform program (per-event
  chunk column counts are compile-time constants shared by all cores).
- Scatter: per 128-hit column, one-hot matmul accumulated into a per-event
  PSUM table [128 lo x 17 q] (sid_p1 = q*128 + lo, sid_p1 in [0, 2049)).
- Per-core partial tables (8 x 2176 entries) are AllReduce-summed across the
  8 cores (68KB).
- Gather: indirect DMA from the reduced table using per-hit flat indices
  (idx = lo*136 + 17*e + q), computed on-device and kept SBUF-resident.
"""

import sys

sys.path.insert(0, "/opt/trn_rl_repo")

import functools

import numpy as np

import concourse.bass as bass
import concourse.mybir as mybir
import concourse.tile as tile
from concourse import bacc
from concourse.bass_utils import run_bass_kernel_spmd

NCORES = 8
B = 8                  # events
SEV = 2049             # sid_p1 range per event
QW = 17                # q = sid_p1 >> 7 in [0, 17)
TBL_COLS = B * QW      # 136
TBL_N = 128 * TBL_COLS  # 17408; flat idx = lo*136 + 17*e + q
BW = 512               # columns per processing block
GW = 16                # columns per one-hot group build
GB = 256               # gather block columns
F32 = mybir.dt.float32
BF16 = mybir.dt.bfloat16
OH_DT = BF16           # dtype of one-hot operands fed to the PE
I32 = mybir.dt.int32
AOP = mybir.AluOpType


def _layout(row_splits):
    rs = np.asarray(row_splits, dtype=np.int64).ravel()
    lens = np.diff(rs)                      # [B]
    pl = -(-lens // NCORES)                 # piece length per core (ceil)
    c_e = -(-pl // 128)                     # columns per chunk (ceil)
    return rs, lens, pl, c_e


@functools.lru_cache(maxsize=4)
def _build(c_e_tuple):
    """Build + compile the uniform per-core program for chunk column counts."""
    c_e = list(c_e_tuple)
    c_total = int(sum(c_e))
    assert c_total > 0

    nc = bacc.Bacc("TRN2", target_bir_lowering=False, debug=False,
                   num_devices=NCORES)
    sid_d = nc.dram_tensor("sid", [128, c_total], I32, kind="ExternalInput").ap()
    corr_d = nc.dram_tensor("corr", [128, c_total], F32, kind="ExternalInput").ap()
    en_d = nc.dram_tensor("energy", [128, c_total], F32, kind="ExternalInput").ap()
    out_d = nc.dram_tensor("out", [8, 16 * c_total], F32, kind="ExternalOutput").ap()
    tloc = nc.dram_tensor("tloc", [1, TBL_N], F32).ap()
    tglob = nc.dram_tensor("tglob", [1, TBL_N], F32).ap()

    with tile.TileContext(nc) as tc:
        with (
            tc.tile_pool(name="const", bufs=1) as constp,
            tc.tile_pool(name="io", bufs=3) as iop,
            tc.tile_pool(name="work", bufs=2) as workp,
            tc.tile_pool(name="oh", bufs=4) as ohp,
            tc.tile_pool(name="og", bufs=2) as ogp,
            tc.tile_pool(name="seg", bufs=1) as segp,
            tc.tile_pool(name="tbl", bufs=1) as tblp,
            tc.tile_pool(name="psum", bufs=1, space="PSUM") as psump,
        ):
            # constants
            iota_lo = constp.tile([128, GW * 128], OH_DT, tag="il_f")
            nc.gpsimd.iota(iota_lo[:], pattern=[[0, GW], [1, 128]], base=0,
                           channel_multiplier=0,
                           allow_small_or_imprecise_dtypes=True)
            iota_q = constp.tile([128, QW], OH_DT, tag="iq_f")
            nc.gpsimd.iota(iota_q[:], pattern=[[1, QW]], base=0,
                           channel_multiplier=0,
                           allow_small_or_imprecise_dtypes=True)
            lo_b = None

            seg_sb = segp.tile([128, c_total], mybir.dt.int16, tag="seg")
            table_sb = tblp.tile([128, TBL_COLS], F32, tag="tbl")
            nc.gpsimd.memset(table_sb[:], 0.0)

            # ---------------- phase A: scatter ----------------
            seg_instrs = []
            col0 = 0
            for e in range(B):
                ce = c_e[e]
                if ce == 0:
                    continue
                psum_e = psump.tile([128, QW], F32, tag=f"ev{e}")
                first = True
                for b0 in range(0, ce, BW):
                    w = min(BW, ce - b0)
                    cs = col0 + b0  # global column offset of this block
                    st = iop.tile([128, BW], I32, tag="sid")
                    ct = iop.tile([128, BW], F32, tag="corr")
                    et = iop.tile([128, BW], F32, tag="en")
                    nc.sync.dma_start(out=st[:, :w], in_=sid_d[:, cs:cs + w])
                    nc.sync.dma_start(out=ct[:, :w], in_=corr_d[:, cs:cs + w])
                    nc.sync.dma_start(out=et[:, :w], in_=en_d[:, cs:cs + w])

                    lo_i = workp.tile([128, BW], I32, tag="lo_i")
                    q_i = workp.tile([128, BW], I32, tag="q_i")
                    lo_f = workp.tile([128, BW], F32, tag="lo_f")
                    lo_b = workp.tile([128, BW], OH_DT, tag="lo_b")
                    q_f = workp.tile([128, BW], F32, tag="q_f")
                    ve = workp.tile([128, BW], F32, tag="ve")
                    v = workp.tile([128, BW], F32, tag="v")

                    vm = workp.tile([128, BW], F32, tag="vm")
                    s_i = workp.tile([128, BW], I32, tag="s_i")
                    nc.vector.tensor_scalar_add(s_i[:, :w], st[:, :w], 1)
                    nc.vector.tensor_scalar(out=lo_i[:, :w], in0=s_i[:, :w],
                                            scalar1=127, scalar2=None,
                                            op0=AOP.bitwise_and)
                    nc.vector.tensor_scalar(out=q_i[:, :w], in0=s_i[:, :w],
                                            scalar1=7, scalar2=None,
                                            op0=AOP.logical_shift_right)
                    nc.vector.tensor_copy(lo_f[:, :w], lo_i[:, :w])
                    nc.scalar.copy(lo_b[:, :w], lo_i[:, :w])
                    nc.vector.tensor_copy(q_f[:, :w], q_i[:, :w])
                    nc.gpsimd.tensor_tensor(out=ve[:, :w], in0=ct[:, :w],
                                            in1=et[:, :w], op=AOP.mult)
                    # v = (sid >= 0) * energy * corr
                    nc.gpsimd.tensor_scalar(out=vm[:, :w], in0=st[:, :w],
                                            scalar1=0, scalar2=None,
                                            op0=AOP.is_ge)
                    nc.gpsimd.tensor_tensor(out=v[:, :w], in0=vm[:, :w],
                                            in1=ve[:, :w], op=AOP.mult)
                    # seg index = lo*136 + q + 17*e  (int16, SBUF-resident)
                    seg32 = s_i
                    nc.vector.tensor_scalar(out=seg32[:, :w],
                                            in0=lo_i[:, :w], scalar1=TBL_COLS,
                                            scalar2=17 * e, op0=AOP.mult,
                                            op1=AOP.add)
                    nc.vector.tensor_tensor(out=seg32[:, :w],
                                            in0=seg32[:, :w],
                                            in1=q_i[:, :w], op=AOP.add)
                    si = nc.vector.tensor_copy(seg_sb[:, cs:cs + w],
                                               seg32[:, :w])
                    seg_instrs.append((cs, cs + w, si))

                    for g0 in range(0, w, GW):
                        gw = min(GW, w - g0)
                        oh = ohp.tile([128, GW * 128], OH_DT, tag="oh")
                        rhs = ohp.tile([128, GW * QW], OH_DT, tag="rhs")
                        nc.vector.tensor_tensor(
                            out=oh[:].rearrange("p (g x) -> p g x", x=128)[:, :gw],
                            in0=lo_b[:, g0:g0 + gw].to_broadcast([128, gw, 128]),
                            in1=iota_lo[:].rearrange("p (g x) -> p g x", x=128)[:, :gw],
                            op=AOP.is_equal)
                        for j in range(gw):
                            c = g0 + j
                            nc.vector.tensor_scalar(
                                out=rhs[:, j * QW:(j + 1) * QW],
                                in0=iota_q[:], scalar1=q_f[:, c:c + 1],
                                scalar2=v[:, c:c + 1], op0=AOP.is_equal,
                                op1=AOP.mult)
                            last = (b0 + c == ce - 1)
                            nc.tensor.matmul(
                                psum_e[:],
                                lhsT=oh[:, j * 128:(j + 1) * 128],
                                rhs=rhs[:, j * QW:(j + 1) * QW],
                                start=first, stop=last)
                            first = False
                nc.vector.tensor_copy(table_sb[:, QW * e:QW * (e + 1)], psum_e[:])
                col0 += ce

            # table -> DRAM (flat order = lo*136 + 17e + q), then AllReduce
            nc.sync.dma_start(out=tloc[:], in_=table_sb[:])
            cc = nc.gpsimd.collective_compute(
                "AllReduce", AOP.add,
                replica_groups=[list(range(NCORES))],
                ins=[tloc[:]], outs=[tglob[:]])

            # ---------------- phase B: gather (Q7 ap_gather) ----------------
            from concourse.tile_rust import add_dep_helper
            tbl_bc = tblp.tile([128, TBL_N], F32, tag="tblbc")
            bi = nc.sync.dma_start(out=tbl_bc[:],
                                   in_=tglob.partition_broadcast(128))
            add_dep_helper(bi.ins, cc.ins, reason="bcast after allreduce")
            for b0 in range(0, c_total, GB):
                w = min(GB, c_total - b0)
                og = ogp.tile([128, 16 * GB], F32, tag="og")
                ag = nc.gpsimd.ap_gather(
                    out_ap=og[:, :16 * w], in_ap=tbl_bc[:],
                    idxs_ap=seg_sb[:, b0:b0 + w],
                    channels=128, num_elems=TBL_N, d=1, num_idxs=16 * w)
                add_dep_helper(ag.ins, bi.ins, reason="gather after bcast")
                for (a0, a1, si) in seg_instrs:
                    if a0 < b0 + w and b0 < a1:
                        add_dep_helper(ag.ins, si.ins, reason="gather after seg")
                # one row per 16-partition group holds the full gathered
                # sequence; ship rows 0,16,...,112 straight to DRAM
                di = nc.sync.dma_start(out=out_d[:, 16 * b0:16 * (b0 + w)],
                                       in_=og[0:128:16, :16 * w])
                add_dep_helper(di.ins, ag.ins, reason="out after gather")

    nc.compile()
    return nc, c_total


class _Runner:
    """Cached PJRT executor for a compiled Bass program (axon path).

    Replicates bass2jax.run_bass_via_pjrt but keeps the jitted callable so
    repeated executions reuse the compiled NEFF, and donates the previous
    call's outputs as the next call's output buffers (kernel writes every
    output element, so zero-init is not required).
    """

    def __init__(self, nc):
        import jax
        from jax.sharding import Mesh, PartitionSpec
        from jax.experimental.shard_map import shard_map
        from concourse import bass2jax
        from concourse.bass2jax import _bass_exec_p, install_neuronx_cc_hook

        install_neuronx_cc_hook()
        self.nc = nc
        self.jax = jax
        partition_name = (nc.partition_id_tensor.name
                          if nc.partition_id_tensor else None)
        in_names, out_names, out_avals = [], [], []
        for alloc in nc.m.functions[0].allocations:
            if not isinstance(alloc, mybir.MemoryLocationSet):
                continue
            name = alloc.memorylocations[0].name
            if alloc.kind == "ExternalInput":
                if name != partition_name:
                    in_names.append(name)
            elif alloc.kind == "ExternalOutput":
                out_names.append(name)
                out_avals.append(jax.core.ShapedArray(
                    tuple(alloc.tensor_shape), mybir.dt.np(alloc.dtype)))
        self.in_names = list(in_names)
        self.out_names = out_names
        self.out_avals = out_avals
        n_params = len(in_names)
        n_outs = len(out_avals)
        all_names = in_names + out_names
        if partition_name is not None:
            all_names = all_names + [partition_name]

        def _body(*args):
            operands = list(args)
            if partition_name is not None:
                operands.append(bass2jax.partition_id_tensor())
            outs = _bass_exec_p.bind(
                *operands,
                out_avals=tuple(out_avals),
                in_names=tuple(all_names),
                out_names=tuple(out_names),
                lowering_input_output_aliases=(),
                sim_require_finite=True,
                sim_require_nnan=True,
                nc=nc,
            )
            return tuple(outs)

        devices = jax.devices()[:NCORES]
        mesh = Mesh(np.asarray(devices), ("core",))
        in_specs = (PartitionSpec("core"),) * (n_params + n_outs)
        out_specs = (PartitionSpec("core"),) * n_outs
        self.donate = tuple(range(n_params, n_params + n_outs))
        self.fn = jax.jit(
            shard_map(_body, mesh=mesh, in_specs=in_specs,
                      out_specs=out_specs, check_rep=False),
            donate_argnums=self.donate, keep_unused=True)
        self._dev_in = None
        self._prev_out = None

    def load_inputs(self, in_maps):
        concat = [
            np.concatenate([np.asarray(in_maps[c][n]) for c in range(NCORES)],
                           axis=0)
            for n in self.in_names
        ]
        self._dev_in = [self.jax.device_put(a) for a in concat]
        self._prev_out = None

    def run_once(self):
        if self._prev_out is None:
            outs = [np.zeros((NCORES * a.shape[0], *a.shape[1:]), a.dtype)
                    for a in self.out_avals]
        else:
            outs = self._prev_out
        res = self.fn(*self._dev_in, *outs)
        self.jax.block_until_ready(res)
        self._prev_out = list(res)
        return res

    def results(self):
        res = self._prev_out
        return [
            {name: np.asarray(res[i]).reshape(NCORES, *self.out_avals[i].shape)[c]
             for i, name in enumerate(self.out_names)}
            for c in range(NCORES)
        ]


_RUNNERS = {}


def _get_runner(c_e_tuple):
    if c_e_tuple not in _RUNNERS:
        nc, c_total = _build(c_e_tuple)
        _RUNNERS[c_e_tuple] = (_Runner(nc), c_total)
    return _RUNNERS[c_e_tuple]


def _prep_core(pred_sid, pred_corr_factor, rechit_energy, rs, pl, c_e, c_total, k):
    sid = np.full((128, c_total), -1, dtype=np.int32)
    corr = np.zeros((128, c_total), dtype=np.float32)
    en = np.zeros((128, c_total), dtype=np.float32)
    col0 = 0
    for e in range(B):
        ce = int(c_e[e])
        if ce == 0:
            continue
        i0 = int(rs[e] + k * pl[e])
        i1 = int(min(rs[e] + (k + 1) * pl[e], rs[e + 1]))
        n = max(0, i1 - i0)
        pad = ce * 128
        if n > 0:
            blk = np.full(pad, -1, dtype=np.int32)
            blk[:n] = pred_sid[i0:i1, 0]
            sid[:, col0:col0 + ce] = blk.reshape(128, ce)
            blkf = np.zeros(pad, dtype=np.float32)
            blkf[:n] = pred_corr_factor[i0:i1, 0]
            corr[:, col0:col0 + ce] = blkf.reshape(128, ce)
            blkf = np.zeros(pad, dtype=np.float32)
            blkf[:n] = rechit_energy[i0:i1, 0]
            en[:, col0:col0 + ce] = blkf.reshape(128, ce)
        col0 += ce
    return {"sid": sid, "corr": corr, "energy": en}


def kernel(pred_sid, pred_corr_factor, rechit_energy, row_splits):
    pred_sid = np.asarray(pred_sid)
    pred_corr_factor = np.asarray(pred_corr_factor)
    rechit_energy = np.asarray(rechit_energy)
    row_splits = np.asarray(row_splits)

    rs, lens, pl, c_e = _layout(row_splits)
    runner, c_total = _get_runner(tuple(int(x) for x in c_e))

    in_maps = [
        _prep_core(pred_sid, pred_corr_factor, rechit_energy, rs, pl, c_e,
                   c_total, k)
        for k in range(NCORES)
    ]
    runner.load_inputs(in_maps)
    runner.run_once()
    results = runner.results()

    n = pred_sid.shape[0]
    out = np.zeros((n,), dtype=np.float32)
    for k in range(NCORES):
        og = results[k]["out"]  # [8, 16*c_total]
        o = og.reshape(8, c_total, 16).transpose(0, 2, 1).reshape(128, c_total)
        col0 = 0
        for e in range(B):
            ce = int(c_e[e])
            if ce == 0:
                continue
            i0 = int(rs[e] + k * pl[e])
            i1 = int(min(rs[e] + (k + 1) * pl[e], rs[e + 1]))
            nk = max(0, i1 - i0)
            if nk > 0:
                out[i0:i1] = o[:, col0:col0 + ce].reshape(-1)[:nk]
            col0 += ce
    return out[:, None]
